# revision 36
# baseline (speedup 1.0000x reference)
"""Trainium2 Bass kernel for nn_AttentionBlock (GroupNorm + QKV + MHA).

Sharding: tensor-parallel over the H=8 heads, one head per NeuronCore.
v3 pipeline (vs v2 baseline at ~179us):
  - GroupNorm stats fully on DVE bn_stats (they hide under the xt DMA
    window); the fold chain is pruned to ~16 serial ops and computes
    rstd = exp(-0.5*ln(var+eps)) so ACT only ever needs the ln/exp
    table set (no Sqrt/Square set switches on the critical path).
  - kq2 ([K;Q], the row-swapped copy of qk1) comes from two SBUF->SBUF
    DMA copies per chunk instead of a second full projection: saves
    ~7us of PE and ~6us of DVE bias work.
  - V^T token-major tiles come from DMA XBAR transposes of vts instead
    of PE transposes + DVE copies: saves ~9us PE + ~6us DVE.
  - mm2 is all-bf16 (fp8 DoubleRow measured no faster than two bf16
    N=512 matmuls on HW, so the fp8 plumbing is dropped).
  - V projection chunks are interleaved into attention chunk 0's mm1
    pair stream, and qk1 projection immediately precedes attention, so
    the exp engines ramp ~25us earlier than v2.
  - softmax exp split across ACT (real Exp) and DVE (Schraudolph
    affine-int16 bitcast) as in v2, 9/7 tiles per chunk.
"""
import sys

try:
    import concourse.bass as bass  # noqa: F401
except ImportError:
    sys.path.insert(0, "/opt/trn_rl_repo")
    import concourse.bass as bass  # noqa: F401

import numpy as np
import concourse.tile as tile
from concourse import bacc, mybir
from concourse.bass_utils import run_bass_kernel_spmd

F32 = mybir.dt.float32
F32R = mybir.dt.float32r
BF16 = mybir.dt.bfloat16
I16 = mybir.dt.int16
FP8 = mybir.dt.float8e4
AF = mybir.ActivationFunctionType
OP = mybir.AluOpType
DR = mybir.MatmulPerfMode.DoubleRow

T = 4096          # tokens (64*64)
C = 512           # channels
H = 8             # heads
CH = C // H       # 64 head dim
G = 32            # groupnorm groups
EPS = 1e-6
QKS = float(CH) ** -0.25   # scale applied to q and k each
NCORES = 8
TC = 512          # t-chunk width
NTC = T // TC     # 8 chunks
NSB = T // 128    # 32 s-blocks
VTP = 80          # vt inner pitch (65 used; 80 keeps dma/DR alignment)

# Schraudolph bf16 exp: bitcast(int16(round(x*SCH_A + SCH_B))) ~= exp(x)/16
SCH_A = 128.0 / float(np.log(2.0))
SCH_B = 16256.0 - 7.0 - 4.0 * 128.0
LN16 = float(np.log(16.0))
# score tiles handled by ACT's real exp (rest go to DVE Schraudolph).
ACT_TILES = frozenset({0, 2, 4, 6, 8, 10, 12, 14, 15})


def build_nc():
    nc = bacc.Bacc()
    xt = nc.declare_dram_parameter("xt", [C, T], BF16, isOutput=False)
    wqk = nc.declare_dram_parameter("wqk", [C, 128], BF16, isOutput=False)
    wv = nc.declare_dram_parameter("wv", [C, CH], BF16, isOutput=False)
    bqk = nc.declare_dram_parameter("bqk", [128, 1], F32, isOutput=False)
    bvT = nc.declare_dram_parameter("bvT", [CH, 1], F32, isOutput=False)
    gns = nc.declare_dram_parameter("gns", [128, 4], F32, isOutput=False)
    gnb = nc.declare_dram_parameter("gnb", [128, 4], F32, isOutput=False)
    ind = nc.declare_dram_parameter("ind", [128, 8], F32R, isOutput=False)
    indt = nc.declare_dram_parameter("indt", [8, 128], F32R, isOutput=False)
    outT = nc.declare_dram_parameter("outT", [CH, T], F32, isOutput=True)

    with tile.TileContext(nc) as tc:
        _build_body(nc, tc, xt, wqk, wv, bqk, bvT, gns, gnb, ind, indt, outT)
    nc.finalize()
    return nc


def _build_body(nc, tc, xt, wqk, wv, bqk, bvT, gns, gnb, ind, indt, outT):
    from contextlib import ExitStack
    ctx = ExitStack()
    with ctx:
        const = ctx.enter_context(tc.tile_pool(name="const", bufs=1))
        big = ctx.enter_context(tc.tile_pool(name="big", bufs=1))
        work = ctx.enter_context(tc.tile_pool(name="work", bufs=2))
        ppool = ctx.enter_context(tc.tile_pool(name="ppool", bufs=6))
        ps_pair = ctx.enter_context(tc.tile_pool(name="ps_pair", bufs=3, space="PSUM"))
        ps_o = ctx.enter_context(tc.tile_pool(name="ps_o", bufs=2, space="PSUM"))
        ps_sm = ps_o

        # ---------------- loads ----------------
        xtk = [big.tile([128, T], BF16, name=f"xt{k}", tag=f"xt{k}")
               for k in range(4)]
        for k in range(4):
            # q3 FIRST: it's the quarter ACT's stats pass consumes, and ACT
            # needs ~2.3us of serial work after it lands -- loading it first
            # pulls the whole stats->fold chain ~5us earlier.
            for q in (3, 0, 1, 2):
                nc.sync.dma_start(out=xtk[k][:, 1024 * q:1024 * (q + 1)],
                                  in_=xt[128 * k:128 * (k + 1),
                                         1024 * q:1024 * (q + 1)])
        wqk_raw, wv_raw = [], []
        for k in range(4):
            a = const.tile([128, 128], BF16, tag=f"wqkr{k}")
            nc.sync.dma_start(out=a, in_=wqk[128 * k:128 * (k + 1), :])
            wqk_raw.append(a)
            v = const.tile([128, CH], BF16, tag=f"wvr{k}")
            nc.sync.dma_start(out=v, in_=wv[128 * k:128 * (k + 1), :])
            wv_raw.append(v)
        bqk_t = const.tile([128, 1], F32, tag="bqk")
        nc.sync.dma_start(out=bqk_t, in_=bqk[:, :])
        bvT_t = const.tile([CH, 1], F32, tag="bvT")
        nc.sync.dma_start(out=bvT_t, in_=bvT[:, :])
        gns_t = const.tile([128, 4], F32, tag="gns")
        nc.sync.dma_start(out=gns_t, in_=gns[:, :])
        gnb_t = const.tile([128, 4], F32, tag="gnb")
        nc.sync.dma_start(out=gnb_t, in_=gnb[:, :])
        ind_t = const.tile([128, 8], F32R, tag="ind")
        nc.sync.dma_start(out=ind_t, in_=ind[:, :])
        indt_t = const.tile([8, 128], F32R, tag="indt")
        nc.sync.dma_start(out=indt_t, in_=indt[:, :])

        # hoist the exp ACT table load to t~0 (dummy op on a const tile)
        ln16b = const.tile([128, 1], F32, tag="ln16b")
        nc.vector.memset(ln16b, -LN16)
        tl_dummy = const.tile([128, 1], F32, tag="tl_dummy")
        nc.scalar.activation(out=tl_dummy, in_=ln16b, func=AF.Exp)

        # PE warmup: HAM needs a DENSE sustained burst to unthrottle (~3.4us
        # of solid activity), so: 8 warmups paced on the k0/k1 quarter
        # arrivals, then 24 back-to-back (dense burst ~t14-19 -> warm PE
        # well before the fold + projections).
        warm_ps = ps_sm.tile([128, TC], F32, name="warm_ps", tag="psO")

        def keep_warm(k=0, q=0, n=1):
            for _ in range(n):
                nc.tensor.matmul(out=warm_ps, lhsT=xtk[k][:, 0:128],
                                 rhs=xtk[k][:, bass.ts(q, 1024)][:, 0:512],
                                 start=True, stop=True)

        for w_i in range(8):
            keep_warm(k=w_i // 4, q=(3, 0, 1, 2)[w_i % 4])
        keep_warm(k=1, q=2, n=24)

        # ---------------- phase A: groupnorm stats -> folded weights ------
        # stats are split between DVE (bn_stats) and ACT (Identity/Square
        # with free-dim accumulate) so they drain within the DMA window.
        # DVE takes 5 quarters' worth per pair of k-tiles via bn_stats; ACT
        # takes q3 for all k plus q2 for k0/k1 (Identity/Square accumulate).
        # This balances the two engines' serial stats time (~13.6us each)
        # inside the xt DMA window; the blend is vectorized across all 4
        # k-tiles with a per-column nD vector.
        RS = work.tile([128, 8], F32R, tag="RS")
        s1 = work.tile([128, 2, 4], F32, tag="s1")
        s2 = work.tile([128, 2, 4], F32, tag="s2")
        nc.vector.memset(s1[:, 0, :], 0.0)
        nc.vector.memset(s2[:, 0, :], 0.0)
        scr = const.tile([128, 1024], BF16, tag="scr")
        mv4 = work.tile([128, 4, 2], F32, tag="mv4")
        for k in range(4):
            act_qs = (2, 3) if k < 2 else (3,)
            dve_qs = [q for q in range(4) if q not in act_qs]
            st = work.tile([128, len(dve_qs) * 2, 6], F32, tag=f"st{k}")
            for j, q in enumerate(dve_qs):
                for h2 in range(2):
                    nc.vector.bn_stats(
                        out=st[:, 2 * j + h2, :],
                        in_=xtk[k][:, 1024 * q + 512 * h2:
                                   1024 * q + 512 * (h2 + 1)])
            for q in act_qs:
                qs = slice(1024 * q, 1024 * (q + 1))
                nc.scalar.activation(out=scr, in_=xtk[k][:, qs],
                                     func=AF.Identity,
                                     accum_out=s1[:, q - 2, k:k + 1])
                nc.scalar.activation(out=scr, in_=xtk[k][:, qs],
                                     func=AF.Square,
                                     accum_out=s2[:, q - 2, k:k + 1])
            nc.vector.bn_aggr(out=mv4[:, k, :], in_=st)
        # dummy exp AFTER the stats Squares: pulls any ACT table-set switch
        # back off the attention critical path (executes in ACT order here)
        nc.scalar.activation(out=tl_dummy, in_=ln16b, func=AF.Exp)
        # vectorized blend over all 4 k-tiles; nD per column = tokens the
        # DVE side covered (2048 for k0/k1, 3072 for k2/k3)
        nDv = const.tile([128, 4], F32, tag="nDv")
        nc.vector.memset(nDv[:, 0:2], 2048.0)
        nc.vector.memset(nDv[:, 2:4], 3072.0)
        t4a = work.tile([128, 4], F32, tag="t4a")
        t4b = work.tile([128, 4], F32, tag="t4b")
        nc.vector.tensor_mul(out=t4a, in0=mv4[:, :, 0], in1=nDv)
        nc.vector.tensor_add(out=t4a, in0=t4a, in1=s1[:, 0, :])
        nc.vector.tensor_add(out=t4a, in0=t4a, in1=s1[:, 1, :])
        nc.vector.tensor_scalar_mul(out=RS[:, 0:8:2], in0=t4a,
                                    scalar1=1.0 / T)
        nc.vector.tensor_mul(out=t4b, in0=mv4[:, :, 0], in1=mv4[:, :, 0])
        nc.vector.tensor_add(out=t4b, in0=t4b, in1=mv4[:, :, 1])
        nc.vector.tensor_mul(out=t4b, in0=t4b, in1=nDv)
        nc.vector.tensor_add(out=t4b, in0=t4b, in1=s2[:, 0, :])
        nc.vector.tensor_add(out=t4b, in0=t4b, in1=s2[:, 1, :])
        nc.vector.tensor_scalar_mul(out=RS[:, 1:8:2], in0=t4b,
                                    scalar1=1.0 / T)
        psG = ps_sm.tile([128, 8], F32, tag="psO")
        nc.tensor.matmul(out=psG[0:8, :], lhsT=ind_t, rhs=RS, start=True,
                         stop=True)
        keep_warm(n=2)  # in-order blip: paces right after psG executes
        # group stats (8 groups-per-tile x 4 tiles); /16 folds the
        # 16-partition indicator sum into the mean. gm lands directly in
        # Gt[:, even]; the last Newton step lands directly in Gt[:, odd].
        Gt = work.tile([8, 8], F32R, tag="Gt")
        gm = Gt[:, 0:8:2]
        nc.vector.tensor_scalar_mul(out=gm, in0=psG[0:8, 0:8:2], scalar1=1.0 / 16)
        gv = work.tile([8, 4], F32, tag="gv")
        t8 = work.tile([8, 4], F32, tag="t8")
        nc.vector.tensor_mul(out=t8, in0=gm, in1=gm)
        nc.vector.tensor_scalar(out=t8, in0=t8, scalar1=-1.0, scalar2=EPS,
                                op0=OP.mult, op1=OP.add)
        nc.vector.tensor_scalar(out=gv, in0=psG[0:8, 1:8:2], scalar1=1.0 / 16,
                                scalar2=None, op0=OP.mult)
        nc.vector.tensor_add(out=gv, in0=gv, in1=t8)
        # rstd = rsqrt(var+eps) fully on DVE: quake-style int32 bit-trick
        # seed + 2 Newton steps (avoids any ACT table-set switch).
        bits_f = work.tile([8, 4], F32, tag="bits_f")
        nc.vector.tensor_copy(out=bits_f, in_=gv.bitcast(mybir.dt.int32))
        seed_f = work.tile([8, 4], F32, tag="seed_f")
        nc.vector.tensor_scalar(out=seed_f, in0=bits_f, scalar1=-0.5,
                                scalar2=1597463007.0, op0=OP.mult, op1=OP.add)
        seed_i = work.tile([8, 4], mybir.dt.int32, tag="seed_i")
        nc.vector.tensor_copy(out=seed_i, in_=seed_f)
        y = work.tile([8, 4], F32, tag="yrs")
        nc.vector.tensor_copy(out=y, in_=seed_i.bitcast(F32))
        tn = work.tile([8, 4], F32, tag="tn")
        nc.vector.tensor_mul(out=tn, in0=y, in1=y)
        nc.vector.tensor_mul(out=tn, in0=tn, in1=gv)
        nc.vector.tensor_scalar(out=tn, in0=tn, scalar1=-0.5,
                                scalar2=1.5, op0=OP.mult, op1=OP.add)
        nc.vector.tensor_mul(out=y, in0=y, in1=tn)
        nc.vector.tensor_mul(out=tn, in0=y, in1=y)
        nc.vector.tensor_mul(out=tn, in0=tn, in1=gv)
        nc.vector.tensor_scalar(out=tn, in0=tn, scalar1=-0.5,
                                scalar2=1.5, op0=OP.mult, op1=OP.add)
        nc.vector.tensor_mul(out=Gt[:, 1:8:2], in0=y, in1=tn)
        psB = ps_sm.tile([128, 8], F32, tag="psO")
        nc.tensor.matmul(out=psB, lhsT=indt_t, rhs=Gt, start=True, stop=True)
        keep_warm(n=2)  # in-order blip: paces right after psB executes
        # per-channel A = rstd*gn_scale ; B = gn_bias - mean*A
        Av = work.tile([128, 4], F32, tag="Av")
        nc.vector.tensor_mul(out=Av, in0=psB[:, 1:8:2], in1=gns_t)
        Aqk = work.tile([128, 4], F32, tag="Aqk")
        nc.vector.tensor_scalar_mul(out=Aqk, in0=Av, scalar1=QKS)
        # scaled weights (qk first: they gate the first projections)
        wqk_s, wv_s = [], []
        for k in range(4):
            a = const.tile([128, 128], BF16, tag=f"wqks{k}")
            nc.vector.tensor_scalar_mul(out=a, in0=wqk_raw[k],
                                        scalar1=Aqk[:, k:k + 1])
            wqk_s.append(a)
        for k in range(4):
            v = const.tile([128, CH], BF16, tag=f"wvs{k}")
            nc.vector.tensor_scalar_mul(out=v, in0=wv_raw[k],
                                        scalar1=Av[:, k:k + 1])
            wv_s.append(v)
        # bias-fold rhs vectors (off the proj critical path)
        t128 = work.tile([128, 4], F32, tag="t128")
        nc.vector.tensor_mul(out=t128, in0=psB[:, 0:8:2], in1=Av)
        Bc = work.tile([128, 4], F32, tag="Bc")
        nc.vector.tensor_sub(out=Bc, in0=gnb_t, in1=t128)
        Br = work.tile([128, 4], BF16, tag="Br")
        nc.vector.tensor_copy(out=Br, in_=Bc)
        Bsr = work.tile([128, 4], BF16, tag="Bsr")
        nc.vector.tensor_scalar_mul(out=Bsr, in0=Bc, scalar1=QKS)

        # ---------------- phase B: projections ----------------
        qk1 = big.tile([128, T], BF16, tag="qk1")   # [Q; K]
        kq2 = big.tile([128, T], BF16, tag="kq2")   # [K; Q] (DMA row-swap)
        vts = big.tile([CH, T], BF16, tag="vts")    # V^T (channel-major)
        # V token-major + ones col: bf16 for DVE-exp'd blocks, fp8e4 paired
        # (DoubleRow) for ACT-exp'd blocks
        vt = big.tile([128, NSB, VTP], BF16, tag="vt")
        nc.vector.memset(vt[:, :, CH:CH + 1], 1.0)
        vt8 = big.tile([128, NSB // 2, 2, VTP], FP8, tag="vt8")
        nc.vector.memset(vt8[:, :, :, CH:CH + 1], 1.0)

        def proj_qk(cc):
            sl = bass.ts(cc, TC)
            ps1 = ps_pair.tile([128, 1024], F32, name=f"qk_{cc}", tag="pp")
            for k in range(4):
                nc.tensor.matmul(out=ps1[:, 0:512], lhsT=wqk_s[k],
                                 rhs=xtk[k][:, sl], start=(k == 0),
                                 stop=(k == 3))
            nc.scalar.activation(out=qk1[:, sl], in_=ps1[:, 0:512],
                                 func=AF.Identity, bias=bqk_dev, scale=1.0)
            # kq2 = qk1 with row halves swapped. SBUF->SBUF copy on the
            # GPSIMD software-DGE queue: separate from the main HW DMA
            # queue (no head-of-line blocking) and costs no PE/DVE/ACT.
            nc.gpsimd.dma_start(out=kq2[0:64, sl], in_=qk1[64:128, sl])
            nc.gpsimd.dma_start(out=kq2[64:128, sl], in_=qk1[0:64, sl])

        # 64x64 identity for the PE transposes of V^T
        ident64 = const.tile([CH, CH], BF16, tag="ident64")
        from concourse.masks import make_identity
        make_identity(nc, ident64)

        def v_chunk(vc):
            # V^T projection chunk + PE transposes into token-major vt
            # (DMA XBAR transposes measured ~17GB/s -- far too slow)
            sl = bass.ts(vc, TC)
            psV = ps_pair.tile([128, 1024], F32, name=f"v_{vc}", tag="pp")
            for k in range(4):
                nc.tensor.matmul(out=psV[0:CH, 0:512], lhsT=wv_s[k],
                                 rhs=xtk[k][:, sl], start=(k == 0),
                                 stop=(k == 3))
            nc.scalar.activation(out=vts[:, sl], in_=psV[0:CH, 0:512],
                                 func=AF.Identity, bias=bv_dev, scale=1.0)
            # 4 PE transposes land in ONE psum tile; DVE then evacuates a
            # tile-pair (2 blocks) per copy instead of one block per copy
            psTrQ = ps_sm.tile([128, 4, CH], BF16, tag="psO")
            for j, ti in enumerate(range(4 * vc, 4 * vc + 4)):
                nc.tensor.transpose(out=psTrQ[:, j, :],
                                    in_=vts[:, bass.ts(ti, 128)],
                                    identity=ident64)
            for t in (2 * vc, 2 * vc + 1):
                j0 = (t - 2 * vc) * 2
                if t in ACT_TILES:
                    nc.vector.tensor_copy(out=vt8[:, t, :, 0:CH],
                                          in_=psTrQ[:, j0:j0 + 2, :])
                else:
                    nc.vector.tensor_copy(out=vt[:, 2 * t:2 * t + 2, 0:CH],
                                          in_=psTrQ[:, j0:j0 + 2, :])

        # first two qk projections before the bias folds (weights are ready
        # earlier than the folded biases; epilogues wait on bqk_dev anyway)
        # folded biases: b'' = s*b_host + (s*B) @ W_raw  (qk) / bv + B @ Wv
        psQKb = ps_sm.tile([128, 1], F32, tag="psO")
        for k in range(4):
            nc.tensor.matmul(out=psQKb, lhsT=wqk_raw[k],
                             rhs=Bsr[:, k:k + 1],
                             start=(k == 0), stop=(k == 3))
        bqk_dev = const.tile([128, 1], F32, tag="bqk_dev")
        nc.vector.tensor_add(out=bqk_dev, in0=psQKb[:, 0:1], in1=bqk_t)
        psBvT = ps_sm.tile([CH, 1], F32, tag="psO")
        for k in range(4):
            nc.tensor.matmul(out=psBvT, lhsT=wv_raw[k],
                             rhs=Br[:, k:k + 1],
                             start=(k == 0), stop=(k == 3))
        bv_dev = const.tile([CH, 1], F32, tag="bv_dev")
        nc.vector.tensor_add(out=bv_dev, in0=psBvT[:, 0:1], in1=bvT_t)

        # interleaved qk1/V projections: qk1 first (it gates attention),
        # V chunks fill the stream so vt transposes land before mm2 needs
        # them. All on a warm PE right after the fold.
        proj_qk(0)
        proj_qk(1)
        for c in range(NTC - 2):
            proj_qk(c + 2)
            v_chunk(c)
        v_chunk(NTC - 2)
        v_chunk(NTC - 1)

        # ---------------- phase C: attention ----------------
        outsb = big.tile([CH, T], F32, tag="outsb")
        NT_C = 16  # tiles of 2 s-blocks; tile == mm1 row-pair

        rb_dram = nc.dram_tensor("rb_dram", [NTC, TC], F32)

        def epi_start(state):
            cc0, psO0 = state
            den = work.tile([1, TC], F32, name=f"den_{cc0}", tag="den")
            nc.vector.tensor_copy(out=den, in_=psO0[CH:CH + 1, :])
            rsm = work.tile([128, 4], F32, name=f"rsm_{cc0}", tag="rsm")
            nc.sync.dma_start(out=rsm, in_=den)
            return rsm

        def epi_finish(state, rsm):
            cc0, psO0 = state
            sl0 = bass.ts(cc0, TC)
            rq = work.tile([128, 4], F32, name=f"rq_{cc0}", tag="rq")
            nc.vector.reciprocal(out=rq, in_=rsm)
            nc.sync.dma_start(out=rb_dram[cc0:cc0 + 1, :], in_=rq)
            row = rb_dram[cc0]
            bc = bass.AP(tensor=row.tensor, offset=row.offset,
                         ap=[[0, CH]] + [list(a) for a in row.ap])
            rb = work.tile([CH, TC], F32, name=f"rb_{cc0}", tag="rb")
            nc.sync.dma_start(out=rb, in_=bc)
            nc.vector.tensor_mul(out=outsb[:, sl0], in0=psO0[0:CH, :], in1=rb)
            nc.sync.dma_start(out=outT[:, 512 * cc0:512 * (cc0 + 1)],
                              in_=outsb[:, sl0])

        def mm1_pair(cc, pr, pp):
            sl = bass.ts(cc, TC)
            for half, blk in ((0, 2 * pr), (1, 2 * pr + 1)):
                dst = pp[:, 512 * half:512 * (half + 1)]
                if half == 0:
                    nc.tensor.matmul(out=dst,
                                     lhsT=kq2[0:64, bass.ts(blk, 128)],
                                     rhs=qk1[0:64, sl], start=True, stop=True,
                                     tile_position=(0, 0))
                else:
                    nc.tensor.matmul(out=dst,
                                     lhsT=qk1[64:128, bass.ts(blk, 128)],
                                     rhs=kq2[64:128, sl], start=True,
                                     stop=True, tile_position=(64, 0))

        def exp_tile(pp, ti):
            if ti in ACT_TILES:
                # exp(x)/16 straight to fp8e4, plane-paired for DoubleRow
                P = ppool.tile([128, 2, 512], FP8, tag="P8")
                nc.scalar.activation(out=P, in_=pp, func=AF.Exp,
                                     bias=ln16b, scale=1.0)
            else:
                P = ppool.tile([128, 1024], BF16, tag="P")
                # per-512 slices: a DVE PSUM read must stay within one bank.
                # bank 1 first so bank 0 frees LAST (see v2 notes).
                # NOTE: splitting one tile's banks across ACT+DVE for
                # balance measured SLOWER (ACT's extra half lands late in
                # the chunk and delays PSUM slot recycling) -- keep 9/7.
                for c in (1, 0):
                    cs = slice(512 * c, 512 * (c + 1))
                    nc.vector.tensor_scalar(out=P[:, cs].bitcast(I16),
                                            in0=pp[:, cs], scalar1=SCH_A,
                                            scalar2=SCH_B, op0=OP.mult,
                                            op1=OP.add)
            return P

        def mm2_tile(psO, ti, P, first, last):
            if ti in ACT_TILES:
                # one DoubleRow matmul covers both s-blocks (K=256 eff.)
                # NOTE: M=128 zero-padded DR weights measured SLOWER (the
                # 256-col LDWEIGHTS doesn't hide) -- keep M=65.
                nc.tensor.matmul(out=psO[0:CH + 1, :],
                                 lhsT=vt8[:, ti, :, 0:CH + 1],
                                 rhs=P, perf_mode=DR,
                                 start=first, stop=last)
                return
            for c in range(2):
                blk = 2 * ti + c
                nc.tensor.matmul(out=psO[0:CH + 1, :],
                                 lhsT=vt[:, blk, 0:CH + 1],
                                 rhs=P[:, 512 * c:512 * (c + 1)],
                                 start=(first and c == 0),
                                 stop=(last and c == 1))

        pending = None
        pending_rsm = None
        for cc in range(NTC):
            psO = ps_o.tile([128, TC], F32, tag="psO")
            # batched streams: 3 mm1 pairs back-to-back, their exps, then
            # the PREVIOUS batch's mm2s. During chunk 0 the V projection
            # chunks are interleaved into the pair stream (one per 2 pairs).
            batch = []        # (ti, pp) awaiting exp
            prev_p = []       # (ti, P) awaiting mm2
            for pr in range(NT_C):
                pp = ps_pair.tile([128, 1024], F32, name=f"pp_{cc}_{pr}",
                                  tag="pp")
                mm1_pair(cc, pr, pp)
                batch.append((pr, pp))
                if pending is not None and pr == 0:
                    pending_rsm = epi_start(pending)
                if len(batch) == 3 or pr == NT_C - 1:
                    newp = [(ti, exp_tile(tp, ti)) for ti, tp in batch]
                    batch = []
                    for ti, P in prev_p:
                        mm2_tile(psO, ti, P, first=(ti == 0), last=False)
                    prev_p = newp
                if pending is not None and pr == 2:
                    epi_finish(pending, pending_rsm)
                    pending = None
                    pending_rsm = None
            for j, (ti, P) in enumerate(prev_p):
                mm2_tile(psO, ti, P, first=False, last=(j == len(prev_p) - 1))
            pending = (cc, psO)
        rsm = epi_start(pending)
        epi_finish(pending, rsm)


_NC_CACHE = None
TRACE = False          # set True (e.g. from test.py) to capture an NTFF profile
TMPDIR = None          # set to a path to keep NEFF/NTFF artifacts
LAST_RESULT = None     # BassKernelResults of the most recent kernel() call


def _get_nc():
    global _NC_CACHE
    if _NC_CACHE is None:
        _NC_CACHE = build_nc()
    return _NC_CACHE


def _to_bf16(a):
    import ml_dtypes
    return np.ascontiguousarray(a).astype(ml_dtypes.bfloat16)


def kernel(x, gn_scale, gn_bias, w_qkv, b_qkv):
    x = np.asarray(x, dtype=np.float32)
    gn_scale = np.asarray(gn_scale, dtype=np.float32)
    gn_bias = np.asarray(gn_bias, dtype=np.float32)
    w_qkv = np.asarray(w_qkv, dtype=np.float32)
    b_qkv = np.asarray(b_qkv, dtype=np.float32)

    spatial = x.shape[:-1]
    xt = _to_bf16(x.reshape(T, C).T)                         # (C, T) bf16
    w3 = w_qkv.reshape(C, C, 3)
    wq, wk, wv = w3[..., 0], w3[..., 1], w3[..., 2]          # (C, C) each
    b3 = b_qkv.reshape(C, 3)
    bq, bk, bvv = b3[:, 0], b3[:, 1], b3[:, 2]

    # indicator matrices: channel partition p -> group j = p // 16
    p_idx = np.arange(128)
    ind = (p_idx[:, None] // 16 == np.arange(8)[None, :]).astype(np.float32)
    indt = np.ascontiguousarray(ind.T)
    gns_dev = np.ascontiguousarray(gn_scale.reshape(4, 128).T)
    gnb_dev = np.ascontiguousarray(gn_bias.reshape(4, 128).T)

    in_maps = []
    for h in range(H):
        hsl = slice(CH * h, CH * (h + 1))
        wq_h, wk_h, wv_h = wq[:, hsl], wk[:, hsl], wv[:, hsl]
        in_maps.append({
            "xt": xt,
            "wqk": _to_bf16(np.concatenate([wq_h, wk_h], axis=1)),
            "wv": _to_bf16(wv_h),
            "bqk": (QKS * np.concatenate([bq[hsl], bk[hsl]])).reshape(128, 1),
            "bvT": bvv[hsl].reshape(CH, 1).copy(),
            "gns": gns_dev,
            "gnb": gnb_dev,
            "ind": ind,
            "indt": indt,
        })

    nc = _get_nc()
    res = run_bass_kernel_spmd(nc, in_maps, list(range(NCORES)), trace=TRACE,
                               tmpdir=TMPDIR)
    global LAST_RESULT
    LAST_RESULT = res
    out = np.empty((T, C), dtype=np.float32)
    for h in range(H):
        out[:, CH * h:CH * (h + 1)] = res.results[h]["outT"].T
    return out.reshape(spatial + (C,))


# revision 40
# speedup vs baseline: 1.1846x; 1.1846x over previous
"""Trainium2 Bass kernel for nn_AttentionBlock (GroupNorm + QKV + MHA).

Sharding: tensor-parallel over the H=8 heads, one head per NeuronCore.
v3 pipeline (vs v2 baseline at ~179us):
  - GroupNorm stats fully on DVE bn_stats (they hide under the xt DMA
    window); the fold chain is pruned to ~16 serial ops and computes
    rstd = exp(-0.5*ln(var+eps)) so ACT only ever needs the ln/exp
    table set (no Sqrt/Square set switches on the critical path).
  - kq2 ([K;Q], the row-swapped copy of qk1) comes from two SBUF->SBUF
    DMA copies per chunk instead of a second full projection: saves
    ~7us of PE and ~6us of DVE bias work.
  - V^T token-major tiles come from DMA XBAR transposes of vts instead
    of PE transposes + DVE copies: saves ~9us PE + ~6us DVE.
  - mm2 is all-bf16 (fp8 DoubleRow measured no faster than two bf16
    N=512 matmuls on HW, so the fp8 plumbing is dropped).
  - V projection chunks are interleaved into attention chunk 0's mm1
    pair stream, and qk1 projection immediately precedes attention, so
    the exp engines ramp ~25us earlier than v2.
  - softmax exp split across ACT (real Exp) and DVE (Schraudolph
    affine-int16 bitcast) as in v2, 9/7 tiles per chunk.
"""
import sys

try:
    import concourse.bass as bass  # noqa: F401
except ImportError:
    sys.path.insert(0, "/opt/trn_rl_repo")
    import concourse.bass as bass  # noqa: F401

import numpy as np
import concourse.tile as tile
from concourse import bacc, mybir
from concourse.bass_utils import run_bass_kernel_spmd

F32 = mybir.dt.float32
F32R = mybir.dt.float32r
BF16 = mybir.dt.bfloat16
I16 = mybir.dt.int16
FP8 = mybir.dt.float8e4
AF = mybir.ActivationFunctionType
OP = mybir.AluOpType
DR = mybir.MatmulPerfMode.DoubleRow

T = 4096          # tokens (64*64)
C = 512           # channels
H = 8             # heads
CH = C // H       # 64 head dim
G = 32            # groupnorm groups
EPS = 1e-6
QKS = float(CH) ** -0.25   # scale applied to q and k each
NCORES = 8
TC = 512          # t-chunk width
NTC = T // TC     # 8 chunks
NSB = T // 128    # 32 s-blocks
VTP = 80          # vt inner pitch (65 used; 80 keeps dma/DR alignment)

# Schraudolph bf16 exp: bitcast(int16(round(x*SCH_A + SCH_B))) ~= exp(x)/16
SCH_A = 128.0 / float(np.log(2.0))
SCH_B = 16256.0 - 7.0 - 4.0 * 128.0
LN16 = float(np.log(16.0))
# score tiles handled by ACT's real exp (rest go to DVE Schraudolph).
ACT_TILES = frozenset({0, 2, 4, 6, 8, 10, 12, 14, 15})


def build_nc():
    nc = bacc.Bacc()
    xt = nc.declare_dram_parameter("xt", [C, T], BF16, isOutput=False)
    wqk = nc.declare_dram_parameter("wqk", [C, 128], BF16, isOutput=False)
    wv = nc.declare_dram_parameter("wv", [C, CH], BF16, isOutput=False)
    bqk = nc.declare_dram_parameter("bqk", [128, 1], F32, isOutput=False)
    bvT = nc.declare_dram_parameter("bvT", [CH, 1], F32, isOutput=False)
    gns = nc.declare_dram_parameter("gns", [128, 4], F32, isOutput=False)
    gnb = nc.declare_dram_parameter("gnb", [128, 4], F32, isOutput=False)
    ind = nc.declare_dram_parameter("ind", [128, 8], F32R, isOutput=False)
    indt = nc.declare_dram_parameter("indt", [8, 128], F32R, isOutput=False)
    outT = nc.declare_dram_parameter("outT", [CH, T], F32, isOutput=True)

    with tile.TileContext(nc) as tc:
        _build_body(nc, tc, xt, wqk, wv, bqk, bvT, gns, gnb, ind, indt, outT)
    nc.finalize()
    return nc


def _build_body(nc, tc, xt, wqk, wv, bqk, bvT, gns, gnb, ind, indt, outT):
    from contextlib import ExitStack
    ctx = ExitStack()
    with ctx:
        const = ctx.enter_context(tc.tile_pool(name="const", bufs=1))
        big = ctx.enter_context(tc.tile_pool(name="big", bufs=1))
        work = ctx.enter_context(tc.tile_pool(name="work", bufs=2))
        ppool = ctx.enter_context(tc.tile_pool(name="ppool", bufs=6))
        ps_pair = ctx.enter_context(tc.tile_pool(name="ps_pair", bufs=3, space="PSUM"))
        ps_o = ctx.enter_context(tc.tile_pool(name="ps_o", bufs=2, space="PSUM"))
        ps_sm = ps_o

        # ---------------- loads ----------------
        xtk = [big.tile([128, T], BF16, name=f"xt{k}", tag=f"xt{k}")
               for k in range(4)]
        for k in range(4):
            # q3 FIRST: it's the quarter ACT's stats pass consumes, and ACT
            # needs ~2.3us of serial work after it lands -- loading it first
            # pulls the whole stats->fold chain ~5us earlier.
            for q in (3, 0, 1, 2):
                nc.sync.dma_start(out=xtk[k][:, 1024 * q:1024 * (q + 1)],
                                  in_=xt[128 * k:128 * (k + 1),
                                         1024 * q:1024 * (q + 1)])
        wqk_raw, wv_raw = [], []
        for k in range(4):
            a = const.tile([128, 128], BF16, tag=f"wqkr{k}")
            nc.sync.dma_start(out=a, in_=wqk[128 * k:128 * (k + 1), :])
            wqk_raw.append(a)
            v = const.tile([128, CH], BF16, tag=f"wvr{k}")
            nc.sync.dma_start(out=v, in_=wv[128 * k:128 * (k + 1), :])
            wv_raw.append(v)
        bqk_t = const.tile([128, 1], F32, tag="bqk")
        nc.sync.dma_start(out=bqk_t, in_=bqk[:, :])
        bvT_t = const.tile([CH, 1], F32, tag="bvT")
        nc.sync.dma_start(out=bvT_t, in_=bvT[:, :])
        gns_t = const.tile([128, 4], F32, tag="gns")
        nc.sync.dma_start(out=gns_t, in_=gns[:, :])
        gnb_t = const.tile([128, 4], F32, tag="gnb")
        nc.sync.dma_start(out=gnb_t, in_=gnb[:, :])
        ind_t = const.tile([128, 8], F32R, tag="ind")
        nc.sync.dma_start(out=ind_t, in_=ind[:, :])
        indt_t = const.tile([8, 128], F32R, tag="indt")
        nc.sync.dma_start(out=indt_t, in_=indt[:, :])

        # hoist the exp ACT table load to t~0 (dummy op on a const tile)
        ln16b = const.tile([128, 1], F32, tag="ln16b")
        nc.vector.memset(ln16b, -LN16)
        tl_dummy = const.tile([128, 1], F32, tag="tl_dummy")
        nc.scalar.activation(out=tl_dummy, in_=ln16b, func=AF.Exp)

        # PE warmup: HAM needs a DENSE sustained burst to unthrottle (~3.4us
        # of solid activity), so: 8 warmups paced on the k0/k1 quarter
        # arrivals, then 24 back-to-back (dense burst ~t14-19 -> warm PE
        # well before the fold + projections).
        warm_ps = ps_sm.tile([128, TC], F32, name="warm_ps", tag="psO")

        def keep_warm(k=0, q=0, n=1):
            for _ in range(n):
                nc.tensor.matmul(out=warm_ps, lhsT=xtk[k][:, 0:128],
                                 rhs=xtk[k][:, bass.ts(q, 1024)][:, 0:512],
                                 start=True, stop=True)

        for w_i in range(8):
            keep_warm(k=w_i // 4, q=(3, 0, 1, 2)[w_i % 4])
        keep_warm(k=1, q=2, n=24)

        # ---------------- phase A: groupnorm stats -> folded weights ------
        # stats are split between DVE (bn_stats) and ACT (Identity/Square
        # with free-dim accumulate) so they drain within the DMA window.
        # DVE takes 5 quarters' worth per pair of k-tiles via bn_stats; ACT
        # takes q3 for all k plus q2 for k0/k1 (Identity/Square accumulate).
        # This balances the two engines' serial stats time (~13.6us each)
        # inside the xt DMA window; the blend is vectorized across all 4
        # k-tiles with a per-column nD vector.
        RS = work.tile([128, 8], F32R, tag="RS")
        s1 = work.tile([128, 2, 4], F32, tag="s1")
        s2 = work.tile([128, 2, 4], F32, tag="s2")
        nc.vector.memset(s1[:, 0, :], 0.0)
        nc.vector.memset(s2[:, 0, :], 0.0)
        scr = const.tile([128, 1024], BF16, tag="scr")
        mv4 = work.tile([128, 4, 2], F32, tag="mv4")
        for k in range(4):
            act_qs = (2, 3) if k < 2 else (3,)
            dve_qs = [q for q in range(4) if q not in act_qs]
            st = work.tile([128, len(dve_qs) * 2, 6], F32, tag=f"st{k}")
            for j, q in enumerate(dve_qs):
                for h2 in range(2):
                    nc.vector.bn_stats(
                        out=st[:, 2 * j + h2, :],
                        in_=xtk[k][:, 1024 * q + 512 * h2:
                                   1024 * q + 512 * (h2 + 1)])
            for q in act_qs:
                qs = slice(1024 * q, 1024 * (q + 1))
                nc.scalar.activation(out=scr, in_=xtk[k][:, qs],
                                     func=AF.Identity,
                                     accum_out=s1[:, q - 2, k:k + 1])
                nc.scalar.activation(out=scr, in_=xtk[k][:, qs],
                                     func=AF.Square,
                                     accum_out=s2[:, q - 2, k:k + 1])
            nc.vector.bn_aggr(out=mv4[:, k, :], in_=st)
        # dummy exp AFTER the stats Squares: pulls any ACT table-set switch
        # back off the attention critical path (executes in ACT order here)
        nc.scalar.activation(out=tl_dummy, in_=ln16b, func=AF.Exp)
        # vectorized blend over all 4 k-tiles; nD per column = tokens the
        # DVE side covered (2048 for k0/k1, 3072 for k2/k3)
        nDv = const.tile([128, 4], F32, tag="nDv")
        nc.vector.memset(nDv[:, 0:2], 2048.0)
        nc.vector.memset(nDv[:, 2:4], 3072.0)
        t4a = work.tile([128, 4], F32, tag="t4a")
        t4b = work.tile([128, 4], F32, tag="t4b")
        nc.vector.tensor_mul(out=t4a, in0=mv4[:, :, 0], in1=nDv)
        nc.vector.tensor_add(out=t4a, in0=t4a, in1=s1[:, 0, :])
        nc.vector.tensor_add(out=t4a, in0=t4a, in1=s1[:, 1, :])
        nc.vector.tensor_scalar_mul(out=RS[:, 0:8:2], in0=t4a,
                                    scalar1=1.0 / T)
        nc.vector.tensor_mul(out=t4b, in0=mv4[:, :, 0], in1=mv4[:, :, 0])
        nc.vector.tensor_add(out=t4b, in0=t4b, in1=mv4[:, :, 1])
        nc.vector.tensor_mul(out=t4b, in0=t4b, in1=nDv)
        nc.vector.tensor_add(out=t4b, in0=t4b, in1=s2[:, 0, :])
        nc.vector.tensor_add(out=t4b, in0=t4b, in1=s2[:, 1, :])
        nc.vector.tensor_scalar_mul(out=RS[:, 1:8:2], in0=t4b,
                                    scalar1=1.0 / T)
        psG = ps_sm.tile([128, 8], F32, tag="psO")
        nc.tensor.matmul(out=psG[0:8, :], lhsT=ind_t, rhs=RS, start=True,
                         stop=True)
        keep_warm(n=2)  # in-order blip: paces right after psG executes
        # group stats (8 groups-per-tile x 4 tiles); /16 folds the
        # 16-partition indicator sum into the mean. gm lands directly in
        # Gt[:, even]; the last Newton step lands directly in Gt[:, odd].
        Gt = work.tile([8, 8], F32R, tag="Gt")
        gm = Gt[:, 0:8:2]
        nc.vector.tensor_scalar_mul(out=gm, in0=psG[0:8, 0:8:2], scalar1=1.0 / 16)
        gv = work.tile([8, 4], F32, tag="gv")
        t8 = work.tile([8, 4], F32, tag="t8")
        nc.vector.tensor_mul(out=t8, in0=gm, in1=gm)
        nc.vector.tensor_scalar(out=t8, in0=t8, scalar1=-1.0, scalar2=EPS,
                                op0=OP.mult, op1=OP.add)
        nc.vector.tensor_scalar(out=gv, in0=psG[0:8, 1:8:2], scalar1=1.0 / 16,
                                scalar2=None, op0=OP.mult)
        nc.vector.tensor_add(out=gv, in0=gv, in1=t8)
        # rstd = rsqrt(var+eps) fully on DVE: quake-style int32 bit-trick
        # seed + 2 Newton steps (avoids any ACT table-set switch).
        bits_f = work.tile([8, 4], F32, tag="bits_f")
        nc.vector.tensor_copy(out=bits_f, in_=gv.bitcast(mybir.dt.int32))
        seed_f = work.tile([8, 4], F32, tag="seed_f")
        nc.vector.tensor_scalar(out=seed_f, in0=bits_f, scalar1=-0.5,
                                scalar2=1597463007.0, op0=OP.mult, op1=OP.add)
        seed_i = work.tile([8, 4], mybir.dt.int32, tag="seed_i")
        nc.vector.tensor_copy(out=seed_i, in_=seed_f)
        y = work.tile([8, 4], F32, tag="yrs")
        nc.vector.tensor_copy(out=y, in_=seed_i.bitcast(F32))
        # single Newton step: seed err ~3.4% -> ~0.18% max after one step,
        # well inside the error budget; saves 4 serial DVE ops on the
        # critical path to the first projection
        tn = work.tile([8, 4], F32, tag="tn")
        nc.vector.tensor_mul(out=tn, in0=y, in1=y)
        nc.vector.tensor_mul(out=tn, in0=tn, in1=gv)
        nc.vector.tensor_scalar(out=tn, in0=tn, scalar1=-0.5,
                                scalar2=1.5, op0=OP.mult, op1=OP.add)
        nc.vector.tensor_mul(out=Gt[:, 1:8:2], in0=y, in1=tn)
        psB = ps_sm.tile([128, 8], F32, tag="psO")
        nc.tensor.matmul(out=psB, lhsT=indt_t, rhs=Gt, start=True, stop=True)
        keep_warm(n=2)  # in-order blip: paces right after psB executes
        # per-channel A = rstd*gn_scale ; B = gn_bias - mean*A
        Av = work.tile([128, 4], F32, tag="Av")
        nc.vector.tensor_mul(out=Av, in0=psB[:, 1:8:2], in1=gns_t)
        Aqk = work.tile([128, 4], F32, tag="Aqk")
        nc.vector.tensor_scalar_mul(out=Aqk, in0=Av, scalar1=QKS)
        # scaled weights (qk first: they gate the first projections)
        # split DVE/ACT so the serial chain before the first projection is
        # half as deep (ACT activation scale= is a per-partition vector)
        wqk_s, wv_s = [], []
        for k in range(4):
            a = const.tile([128, 128], BF16, tag=f"wqks{k}")
            if k < 2:
                nc.vector.tensor_scalar_mul(out=a, in0=wqk_raw[k],
                                            scalar1=Aqk[:, k:k + 1])
            else:
                nc.scalar.activation(out=a, in_=wqk_raw[k], func=AF.Identity,
                                     scale=Aqk[:, k:k + 1])
            wqk_s.append(a)
        for k in range(4):
            v = const.tile([128, CH], BF16, tag=f"wvs{k}")
            nc.scalar.activation(out=v, in_=wv_raw[k], func=AF.Identity,
                                 scale=Av[:, k:k + 1])
            wv_s.append(v)
        # bias-fold rhs vectors (off the proj critical path)
        t128 = work.tile([128, 4], F32, tag="t128")
        nc.vector.tensor_mul(out=t128, in0=psB[:, 0:8:2], in1=Av)
        Bc = work.tile([128, 4], F32, tag="Bc")
        nc.vector.tensor_sub(out=Bc, in0=gnb_t, in1=t128)
        Br = work.tile([128, 4], BF16, tag="Br")
        nc.vector.tensor_copy(out=Br, in_=Bc)
        Bsr = work.tile([128, 4], BF16, tag="Bsr")
        nc.vector.tensor_scalar_mul(out=Bsr, in0=Bc, scalar1=QKS)

        # ---------------- phase B: projections ----------------
        qk1 = big.tile([128, T], BF16, tag="qk1")   # [Q; K]
        kq2 = big.tile([128, T], BF16, tag="kq2")   # [K; Q] (DMA row-swap)
        vts = big.tile([CH, T], BF16, tag="vts")    # V^T (channel-major)
        # V token-major + ones col: bf16 for DVE-exp'd blocks, fp8e4 paired
        # (DoubleRow) for ACT-exp'd blocks
        vt = big.tile([128, NSB, VTP], BF16, tag="vt")
        nc.vector.memset(vt[:, :, CH:CH + 1], 1.0)
        vt8 = big.tile([128, NSB // 2, 2, VTP], FP8, tag="vt8")
        nc.vector.memset(vt8[:, :, :, CH:CH + 1], 1.0)

        def proj_qk(cc):
            sl = bass.ts(cc, TC)
            ps1 = ps_pair.tile([128, 1024], F32, name=f"qk_{cc}", tag="pp")
            for k in range(4):
                nc.tensor.matmul(out=ps1[:, 0:512], lhsT=wqk_s[k],
                                 rhs=xtk[k][:, sl], start=(k == 0),
                                 stop=(k == 3))
            nc.scalar.activation(out=qk1[:, sl], in_=ps1[:, 0:512],
                                 func=AF.Identity, bias=bqk_dev, scale=1.0)
            # kq2 = qk1 with row halves swapped. SBUF->SBUF copy on the
            # GPSIMD software-DGE queue: separate from the main HW DMA
            # queue (no head-of-line blocking) and costs no PE/DVE/ACT.
            nc.gpsimd.dma_start(out=kq2[0:64, sl], in_=qk1[64:128, sl])
            nc.gpsimd.dma_start(out=kq2[64:128, sl], in_=qk1[0:64, sl])

        # 64x64 identity for the PE transposes of V^T
        ident64 = const.tile([CH, CH], BF16, tag="ident64")
        from concourse.masks import make_identity
        make_identity(nc, ident64)

        def v_chunk(vc):
            # V^T projection chunk + PE transposes into token-major vt
            # (DMA XBAR transposes measured ~17GB/s -- far too slow)
            sl = bass.ts(vc, TC)
            psV = ps_pair.tile([128, 1024], F32, name=f"v_{vc}", tag="pp")
            for k in range(4):
                nc.tensor.matmul(out=psV[0:CH, 0:512], lhsT=wv_s[k],
                                 rhs=xtk[k][:, sl], start=(k == 0),
                                 stop=(k == 3))
            nc.scalar.activation(out=vts[:, sl], in_=psV[0:CH, 0:512],
                                 func=AF.Identity, bias=bv_dev, scale=1.0)
            # 4 PE transposes land in ONE psum tile; DVE then evacuates a
            # tile-pair (2 blocks) per copy instead of one block per copy
            psTrQ = ps_sm.tile([128, 4, CH], BF16, tag="psO")
            for j, ti in enumerate(range(4 * vc, 4 * vc + 4)):
                nc.tensor.transpose(out=psTrQ[:, j, :],
                                    in_=vts[:, bass.ts(ti, 128)],
                                    identity=ident64)
            for t in (2 * vc, 2 * vc + 1):
                j0 = (t - 2 * vc) * 2
                if t in ACT_TILES:
                    nc.vector.tensor_copy(out=vt8[:, t, :, 0:CH],
                                          in_=psTrQ[:, j0:j0 + 2, :])
                else:
                    nc.vector.tensor_copy(out=vt[:, 2 * t:2 * t + 2, 0:CH],
                                          in_=psTrQ[:, j0:j0 + 2, :])

        # first two qk projections before the bias folds (weights are ready
        # earlier than the folded biases; epilogues wait on bqk_dev anyway)
        # folded biases: b'' = s*b_host + (s*B) @ W_raw  (qk) / bv + B @ Wv
        psQKb = ps_sm.tile([128, 1], F32, tag="psO")
        for k in range(4):
            nc.tensor.matmul(out=psQKb, lhsT=wqk_raw[k],
                             rhs=Bsr[:, k:k + 1],
                             start=(k == 0), stop=(k == 3))
        bqk_dev = const.tile([128, 1], F32, tag="bqk_dev")
        nc.vector.tensor_add(out=bqk_dev, in0=psQKb[:, 0:1], in1=bqk_t)
        psBvT = ps_sm.tile([CH, 1], F32, tag="psO")
        for k in range(4):
            nc.tensor.matmul(out=psBvT, lhsT=wv_raw[k],
                             rhs=Br[:, k:k + 1],
                             start=(k == 0), stop=(k == 3))
        bv_dev = const.tile([CH, 1], F32, tag="bv_dev")
        nc.vector.tensor_add(out=bv_dev, in0=psBvT[:, 0:1], in1=bvT_t)

        # interleaved qk1/V projections: qk1 first (it gates attention),
        # V chunks fill the stream so vt transposes land before mm2 needs
        # them. All on a warm PE right after the fold.
        proj_qk(0)
        proj_qk(1)
        for c in range(NTC - 2):
            proj_qk(c + 2)
            v_chunk(c)
        v_chunk(NTC - 2)
        v_chunk(NTC - 1)

        # ---------------- phase C: attention ----------------
        outsb = big.tile([CH, T], F32, tag="outsb")
        NT_C = 16  # tiles of 2 s-blocks; tile == mm1 row-pair

        rb_dram = nc.dram_tensor("rb_dram", [NTC, TC], F32)

        def epi_start(state):
            cc0, psO0 = state
            den = work.tile([1, TC], F32, name=f"den_{cc0}", tag="den")
            nc.vector.tensor_copy(out=den, in_=psO0[CH:CH + 1, :])
            rsm = work.tile([128, 4], F32, name=f"rsm_{cc0}", tag="rsm")
            nc.sync.dma_start(out=rsm, in_=den)
            return rsm

        def epi_finish(state, rsm):
            cc0, psO0 = state
            sl0 = bass.ts(cc0, TC)
            rq = work.tile([128, 4], F32, name=f"rq_{cc0}", tag="rq")
            nc.vector.reciprocal(out=rq, in_=rsm)
            nc.sync.dma_start(out=rb_dram[cc0:cc0 + 1, :], in_=rq)
            row = rb_dram[cc0]
            bc = bass.AP(tensor=row.tensor, offset=row.offset,
                         ap=[[0, CH]] + [list(a) for a in row.ap])
            rb = work.tile([CH, TC], F32, name=f"rb_{cc0}", tag="rb")
            nc.sync.dma_start(out=rb, in_=bc)
            nc.vector.tensor_mul(out=outsb[:, sl0], in0=psO0[0:CH, :], in1=rb)
            nc.sync.dma_start(out=outT[:, 512 * cc0:512 * (cc0 + 1)],
                              in_=outsb[:, sl0])

        def mm1_pair(cc, pr, pp):
            sl = bass.ts(cc, TC)
            for half, blk in ((0, 2 * pr), (1, 2 * pr + 1)):
                dst = pp[:, 512 * half:512 * (half + 1)]
                if half == 0:
                    nc.tensor.matmul(out=dst,
                                     lhsT=kq2[0:64, bass.ts(blk, 128)],
                                     rhs=qk1[0:64, sl], start=True, stop=True,
                                     tile_position=(0, 0))
                else:
                    nc.tensor.matmul(out=dst,
                                     lhsT=qk1[64:128, bass.ts(blk, 128)],
                                     rhs=kq2[64:128, sl], start=True,
                                     stop=True, tile_position=(64, 0))

        def exp_tile(pp, ti):
            if ti in ACT_TILES:
                # exp(x)/16 straight to fp8e4, plane-paired for DoubleRow
                P = ppool.tile([128, 2, 512], FP8, tag="P8")
                nc.scalar.activation(out=P, in_=pp, func=AF.Exp,
                                     bias=ln16b, scale=1.0)
            else:
                P = ppool.tile([128, 1024], BF16, tag="P")
                # per-512 slices: a DVE PSUM read must stay within one bank.
                # bank 1 first so bank 0 frees LAST (see v2 notes).
                # NOTE: splitting one tile's banks across ACT+DVE for
                # balance measured SLOWER (ACT's extra half lands late in
                # the chunk and delays PSUM slot recycling) -- keep 9/7.
                for c in (1, 0):
                    cs = slice(512 * c, 512 * (c + 1))
                    nc.vector.tensor_scalar(out=P[:, cs].bitcast(I16),
                                            in0=pp[:, cs], scalar1=SCH_A,
                                            scalar2=SCH_B, op0=OP.mult,
                                            op1=OP.add)
            return P

        def mm2_tile(psO, ti, P, first, last):
            if ti in ACT_TILES:
                # one DoubleRow matmul covers both s-blocks (K=256 eff.)
                # NOTE: M=128 zero-padded DR weights measured SLOWER (the
                # 256-col LDWEIGHTS doesn't hide) -- keep M=65.
                nc.tensor.matmul(out=psO[0:CH + 1, :],
                                 lhsT=vt8[:, ti, :, 0:CH + 1],
                                 rhs=P, perf_mode=DR,
                                 start=first, stop=last)
                return
            for c in range(2):
                blk = 2 * ti + c
                nc.tensor.matmul(out=psO[0:CH + 1, :],
                                 lhsT=vt[:, blk, 0:CH + 1],
                                 rhs=P[:, 512 * c:512 * (c + 1)],
                                 start=(first and c == 0),
                                 stop=(last and c == 1))

        pending = None
        pending_rsm = None
        for cc in range(NTC):
            psO = ps_o.tile([128, TC], F32, tag="psO")
            # batched streams: 3 mm1 pairs back-to-back, their exps, then
            # the PREVIOUS batch's mm2s. During chunk 0 the V projection
            # chunks are interleaved into the pair stream (one per 2 pairs).
            batch = []        # (ti, pp) awaiting exp
            prev_p = []       # (ti, P) awaiting mm2
            for pr in range(NT_C):
                pp = ps_pair.tile([128, 1024], F32, name=f"pp_{cc}_{pr}",
                                  tag="pp")
                mm1_pair(cc, pr, pp)
                batch.append((pr, pp))
                if pending is not None and pr == 0:
                    pending_rsm = epi_start(pending)
                if len(batch) == 3 or pr == NT_C - 1:
                    newp = [(ti, exp_tile(tp, ti)) for ti, tp in batch]
                    batch = []
                    for ti, P in prev_p:
                        mm2_tile(psO, ti, P, first=(ti == 0), last=False)
                    prev_p = newp
                if pending is not None and pr == 2:
                    epi_finish(pending, pending_rsm)
                    pending = None
                    pending_rsm = None
            for j, (ti, P) in enumerate(prev_p):
                mm2_tile(psO, ti, P, first=False, last=(j == len(prev_p) - 1))
            pending = (cc, psO)
        rsm = epi_start(pending)
        epi_finish(pending, rsm)


_NC_CACHE = None
TRACE = False          # set True (e.g. from test.py) to capture an NTFF profile
TMPDIR = None          # set to a path to keep NEFF/NTFF artifacts
LAST_RESULT = None     # BassKernelResults of the most recent kernel() call


def _get_nc():
    global _NC_CACHE
    if _NC_CACHE is None:
        _NC_CACHE = build_nc()
    return _NC_CACHE


def _to_bf16(a):
    import ml_dtypes
    return np.ascontiguousarray(a).astype(ml_dtypes.bfloat16)


def kernel(x, gn_scale, gn_bias, w_qkv, b_qkv):
    x = np.asarray(x, dtype=np.float32)
    gn_scale = np.asarray(gn_scale, dtype=np.float32)
    gn_bias = np.asarray(gn_bias, dtype=np.float32)
    w_qkv = np.asarray(w_qkv, dtype=np.float32)
    b_qkv = np.asarray(b_qkv, dtype=np.float32)

    spatial = x.shape[:-1]
    xt = _to_bf16(x.reshape(T, C).T)                         # (C, T) bf16
    w3 = w_qkv.reshape(C, C, 3)
    wq, wk, wv = w3[..., 0], w3[..., 1], w3[..., 2]          # (C, C) each
    b3 = b_qkv.reshape(C, 3)
    bq, bk, bvv = b3[:, 0], b3[:, 1], b3[:, 2]

    # indicator matrices: channel partition p -> group j = p // 16
    p_idx = np.arange(128)
    ind = (p_idx[:, None] // 16 == np.arange(8)[None, :]).astype(np.float32)
    indt = np.ascontiguousarray(ind.T)
    gns_dev = np.ascontiguousarray(gn_scale.reshape(4, 128).T)
    gnb_dev = np.ascontiguousarray(gn_bias.reshape(4, 128).T)

    in_maps = []
    for h in range(H):
        hsl = slice(CH * h, CH * (h + 1))
        wq_h, wk_h, wv_h = wq[:, hsl], wk[:, hsl], wv[:, hsl]
        in_maps.append({
            "xt": xt,
            "wqk": _to_bf16(np.concatenate([wq_h, wk_h], axis=1)),
            "wv": _to_bf16(wv_h),
            "bqk": (QKS * np.concatenate([bq[hsl], bk[hsl]])).reshape(128, 1),
            "bvT": bvv[hsl].reshape(CH, 1).copy(),
            "gns": gns_dev,
            "gnb": gnb_dev,
            "ind": ind,
            "indt": indt,
        })

    nc = _get_nc()
    res = run_bass_kernel_spmd(nc, in_maps, list(range(NCORES)), trace=TRACE,
                               tmpdir=TMPDIR)
    global LAST_RESULT
    LAST_RESULT = res
    out = np.empty((T, C), dtype=np.float32)
    for h in range(H):
        out[:, CH * h:CH * (h + 1)] = res.results[h]["outT"].T
    return out.reshape(spatial + (C,))


# revision 41
# speedup vs baseline: 1.2061x; 1.0182x over previous
"""Trainium2 Bass kernel for nn_AttentionBlock (GroupNorm + QKV + MHA).

Sharding: tensor-parallel over the H=8 heads, one head per NeuronCore.
v3 pipeline (vs v2 baseline at ~179us):
  - GroupNorm stats fully on DVE bn_stats (they hide under the xt DMA
    window); the fold chain is pruned to ~16 serial ops and computes
    rstd = exp(-0.5*ln(var+eps)) so ACT only ever needs the ln/exp
    table set (no Sqrt/Square set switches on the critical path).
  - kq2 ([K;Q], the row-swapped copy of qk1) comes from two SBUF->SBUF
    DMA copies per chunk instead of a second full projection: saves
    ~7us of PE and ~6us of DVE bias work.
  - V^T token-major tiles come from DMA XBAR transposes of vts instead
    of PE transposes + DVE copies: saves ~9us PE + ~6us DVE.
  - mm2 is all-bf16 (fp8 DoubleRow measured no faster than two bf16
    N=512 matmuls on HW, so the fp8 plumbing is dropped).
  - V projection chunks are interleaved into attention chunk 0's mm1
    pair stream, and qk1 projection immediately precedes attention, so
    the exp engines ramp ~25us earlier than v2.
  - softmax exp split across ACT (real Exp) and DVE (Schraudolph
    affine-int16 bitcast) as in v2, 9/7 tiles per chunk.
"""
import sys

try:
    import concourse.bass as bass  # noqa: F401
except ImportError:
    sys.path.insert(0, "/opt/trn_rl_repo")
    import concourse.bass as bass  # noqa: F401

import numpy as np
import concourse.tile as tile
from concourse import bacc, mybir
from concourse.bass_utils import run_bass_kernel_spmd

F32 = mybir.dt.float32
F32R = mybir.dt.float32r
BF16 = mybir.dt.bfloat16
I16 = mybir.dt.int16
FP8 = mybir.dt.float8e4
AF = mybir.ActivationFunctionType
OP = mybir.AluOpType
DR = mybir.MatmulPerfMode.DoubleRow

T = 4096          # tokens (64*64)
C = 512           # channels
H = 8             # heads
CH = C // H       # 64 head dim
G = 32            # groupnorm groups
EPS = 1e-6
QKS = float(CH) ** -0.25   # scale applied to q and k each
NCORES = 8
TC = 512          # t-chunk width
NTC = T // TC     # 8 chunks
NSB = T // 128    # 32 s-blocks
VTP = 80          # vt inner pitch (65 used; 80 keeps dma/DR alignment)

# Schraudolph bf16 exp: bitcast(int16(round(x*SCH_A + SCH_B))) ~= exp(x)/16
SCH_A = 128.0 / float(np.log(2.0))
SCH_B = 16256.0 - 7.0 - 4.0 * 128.0
LN16 = float(np.log(16.0))
# score tiles handled by ACT's real exp (rest go to DVE Schraudolph).
ACT_TILES = frozenset({0, 2, 4, 6, 8, 10, 12, 14, 15})


def build_nc():
    nc = bacc.Bacc()
    xt = nc.declare_dram_parameter("xt", [C, T], BF16, isOutput=False)
    wqk = nc.declare_dram_parameter("wqk", [C, 128], BF16, isOutput=False)
    wv = nc.declare_dram_parameter("wv", [C, CH], BF16, isOutput=False)
    bqk = nc.declare_dram_parameter("bqk", [128, 1], F32, isOutput=False)
    bvT = nc.declare_dram_parameter("bvT", [CH, 1], F32, isOutput=False)
    gns = nc.declare_dram_parameter("gns", [128, 4], F32, isOutput=False)
    gnb = nc.declare_dram_parameter("gnb", [128, 4], F32, isOutput=False)
    ind = nc.declare_dram_parameter("ind", [128, 8], F32R, isOutput=False)
    indt = nc.declare_dram_parameter("indt", [8, 128], F32R, isOutput=False)
    outT = nc.declare_dram_parameter("outT", [CH, T], F32, isOutput=True)

    with tile.TileContext(nc) as tc:
        _build_body(nc, tc, xt, wqk, wv, bqk, bvT, gns, gnb, ind, indt, outT)
    nc.finalize()
    return nc


def _build_body(nc, tc, xt, wqk, wv, bqk, bvT, gns, gnb, ind, indt, outT):
    from contextlib import ExitStack
    ctx = ExitStack()
    with ctx:
        const = ctx.enter_context(tc.tile_pool(name="const", bufs=1))
        big = ctx.enter_context(tc.tile_pool(name="big", bufs=1))
        work = ctx.enter_context(tc.tile_pool(name="work", bufs=2))
        ppool = ctx.enter_context(tc.tile_pool(name="ppool", bufs=6))
        ps_pair = ctx.enter_context(tc.tile_pool(name="ps_pair", bufs=3, space="PSUM"))
        ps_o = ctx.enter_context(tc.tile_pool(name="ps_o", bufs=2, space="PSUM"))
        ps_sm = ps_o

        # ---------------- loads ----------------
        xtk = [big.tile([128, T], BF16, name=f"xt{k}", tag=f"xt{k}")
               for k in range(4)]
        for k in range(4):
            # q3 FIRST: it's the quarter ACT's stats pass consumes, and ACT
            # needs ~2.3us of serial work after it lands -- loading it first
            # pulls the whole stats->fold chain ~5us earlier.
            for q in (3, 0, 1, 2):
                nc.sync.dma_start(out=xtk[k][:, 1024 * q:1024 * (q + 1)],
                                  in_=xt[128 * k:128 * (k + 1),
                                         1024 * q:1024 * (q + 1)])
        wqk_raw, wv_raw = [], []
        for k in range(4):
            a = const.tile([128, 128], BF16, tag=f"wqkr{k}")
            nc.sync.dma_start(out=a, in_=wqk[128 * k:128 * (k + 1), :])
            wqk_raw.append(a)
            v = const.tile([128, CH], BF16, tag=f"wvr{k}")
            nc.sync.dma_start(out=v, in_=wv[128 * k:128 * (k + 1), :])
            wv_raw.append(v)
        bqk_t = const.tile([128, 1], F32, tag="bqk")
        nc.sync.dma_start(out=bqk_t, in_=bqk[:, :])
        bvT_t = const.tile([CH, 1], F32, tag="bvT")
        nc.sync.dma_start(out=bvT_t, in_=bvT[:, :])
        gns_t = const.tile([128, 4], F32, tag="gns")
        nc.sync.dma_start(out=gns_t, in_=gns[:, :])
        gnb_t = const.tile([128, 4], F32, tag="gnb")
        nc.sync.dma_start(out=gnb_t, in_=gnb[:, :])
        ind_t = const.tile([128, 8], F32R, tag="ind")
        nc.sync.dma_start(out=ind_t, in_=ind[:, :])
        indt_t = const.tile([8, 128], F32R, tag="indt")
        nc.sync.dma_start(out=indt_t, in_=indt[:, :])

        # hoist the exp ACT table load to t~0 (dummy op on a const tile)
        ln16b = const.tile([128, 1], F32, tag="ln16b")
        nc.vector.memset(ln16b, -LN16)
        tl_dummy = const.tile([128, 1], F32, tag="tl_dummy")
        nc.scalar.activation(out=tl_dummy, in_=ln16b, func=AF.Exp)

        # PE warmup: HAM needs a DENSE sustained burst to unthrottle (~3.4us
        # of solid activity), so: 8 warmups paced on the k0/k1 quarter
        # arrivals, then 24 back-to-back (dense burst ~t14-19 -> warm PE
        # well before the fold + projections).
        warm_ps = ps_sm.tile([128, TC], F32, name="warm_ps", tag="psO")

        def keep_warm(k=0, q=0, n=1):
            for _ in range(n):
                nc.tensor.matmul(out=warm_ps, lhsT=xtk[k][:, 0:128],
                                 rhs=xtk[k][:, bass.ts(q, 1024)][:, 0:512],
                                 start=True, stop=True)

        for w_i in range(8):
            keep_warm(k=w_i // 4, q=(3, 0, 1, 2)[w_i % 4])
        keep_warm(k=1, q=2, n=24)

        # ---------------- phase A: groupnorm stats -> folded weights ------
        # stats are split between DVE (bn_stats) and ACT (Identity/Square
        # with free-dim accumulate) so they drain within the DMA window.
        # DVE takes 5 quarters' worth per pair of k-tiles via bn_stats; ACT
        # takes q3 for all k plus q2 for k0/k1 (Identity/Square accumulate).
        # This balances the two engines' serial stats time (~13.6us each)
        # inside the xt DMA window; the blend is vectorized across all 4
        # k-tiles with a per-column nD vector.
        RS = work.tile([128, 8], F32R, tag="RS")
        s1 = work.tile([128, 2, 4], F32, tag="s1")
        s2 = work.tile([128, 2, 4], F32, tag="s2")
        nc.vector.memset(s1[:, 0, :], 0.0)
        nc.vector.memset(s2[:, 0, :], 0.0)
        scr = const.tile([128, 1024], BF16, tag="scr")
        mv4 = work.tile([128, 4, 2], F32, tag="mv4")
        for k in range(4):
            act_qs = (2, 3) if k < 2 else (3,)
            dve_qs = [q for q in range(4) if q not in act_qs]
            st = work.tile([128, len(dve_qs) * 2, 6], F32, tag=f"st{k}")
            for j, q in enumerate(dve_qs):
                for h2 in range(2):
                    nc.vector.bn_stats(
                        out=st[:, 2 * j + h2, :],
                        in_=xtk[k][:, 1024 * q + 512 * h2:
                                   1024 * q + 512 * (h2 + 1)])
            for q in act_qs:
                qs = slice(1024 * q, 1024 * (q + 1))
                nc.scalar.activation(out=scr, in_=xtk[k][:, qs],
                                     func=AF.Identity,
                                     accum_out=s1[:, q - 2, k:k + 1])
                nc.scalar.activation(out=scr, in_=xtk[k][:, qs],
                                     func=AF.Square,
                                     accum_out=s2[:, q - 2, k:k + 1])
            nc.vector.bn_aggr(out=mv4[:, k, :], in_=st)
        # dummy exp AFTER the stats Squares: pulls any ACT table-set switch
        # back off the attention critical path (executes in ACT order here)
        nc.scalar.activation(out=tl_dummy, in_=ln16b, func=AF.Exp)
        # vectorized blend over all 4 k-tiles; nD per column = tokens the
        # DVE side covered (2048 for k0/k1, 3072 for k2/k3)
        nDv = const.tile([128, 4], F32, tag="nDv")
        nc.vector.memset(nDv[:, 0:2], 2048.0)
        nc.vector.memset(nDv[:, 2:4], 3072.0)
        t4a = work.tile([128, 4], F32, tag="t4a")
        t4b = work.tile([128, 4], F32, tag="t4b")
        nc.vector.tensor_mul(out=t4a, in0=mv4[:, :, 0], in1=nDv)
        nc.vector.tensor_add(out=t4a, in0=t4a, in1=s1[:, 0, :])
        nc.vector.tensor_add(out=t4a, in0=t4a, in1=s1[:, 1, :])
        nc.vector.tensor_scalar_mul(out=RS[:, 0:8:2], in0=t4a,
                                    scalar1=1.0 / T)
        nc.vector.tensor_mul(out=t4b, in0=mv4[:, :, 0], in1=mv4[:, :, 0])
        nc.vector.tensor_add(out=t4b, in0=t4b, in1=mv4[:, :, 1])
        nc.vector.tensor_mul(out=t4b, in0=t4b, in1=nDv)
        nc.vector.tensor_add(out=t4b, in0=t4b, in1=s2[:, 0, :])
        nc.vector.tensor_add(out=t4b, in0=t4b, in1=s2[:, 1, :])
        nc.vector.tensor_scalar_mul(out=RS[:, 1:8:2], in0=t4b,
                                    scalar1=1.0 / T)
        psG = ps_sm.tile([128, 8], F32, tag="psO")
        nc.tensor.matmul(out=psG[0:8, :], lhsT=ind_t, rhs=RS, start=True,
                         stop=True)
        keep_warm(n=2)  # in-order blip: paces right after psG executes
        # group stats (8 groups-per-tile x 4 tiles); /16 folds the
        # 16-partition indicator sum into the mean. gm lands directly in
        # Gt[:, even]; the last Newton step lands directly in Gt[:, odd].
        Gt = work.tile([8, 8], F32R, tag="Gt")
        gm = Gt[:, 0:8:2]
        nc.vector.tensor_scalar_mul(out=gm, in0=psG[0:8, 0:8:2], scalar1=1.0 / 16)
        gv = work.tile([8, 4], F32, tag="gv")
        t8 = work.tile([8, 4], F32, tag="t8")
        nc.vector.tensor_mul(out=t8, in0=gm, in1=gm)
        nc.vector.tensor_scalar(out=t8, in0=t8, scalar1=-1.0, scalar2=EPS,
                                op0=OP.mult, op1=OP.add)
        nc.vector.tensor_scalar(out=gv, in0=psG[0:8, 1:8:2], scalar1=1.0 / 16,
                                scalar2=None, op0=OP.mult)
        nc.vector.tensor_add(out=gv, in0=gv, in1=t8)
        # rstd = rsqrt(var+eps) fully on DVE: quake-style int32 bit-trick
        # seed + 2 Newton steps (avoids any ACT table-set switch).
        bits_f = work.tile([8, 4], F32, tag="bits_f")
        nc.vector.tensor_copy(out=bits_f, in_=gv.bitcast(mybir.dt.int32))
        seed_f = work.tile([8, 4], F32, tag="seed_f")
        nc.vector.tensor_scalar(out=seed_f, in0=bits_f, scalar1=-0.5,
                                scalar2=1597463007.0, op0=OP.mult, op1=OP.add)
        seed_i = work.tile([8, 4], mybir.dt.int32, tag="seed_i")
        nc.vector.tensor_copy(out=seed_i, in_=seed_f)
        y = work.tile([8, 4], F32, tag="yrs")
        nc.vector.tensor_copy(out=y, in_=seed_i.bitcast(F32))
        # single Newton step: seed err ~3.4% -> ~0.18% max after one step,
        # well inside the error budget; saves 4 serial DVE ops on the
        # critical path to the first projection
        tn = work.tile([8, 4], F32, tag="tn")
        nc.vector.tensor_mul(out=tn, in0=y, in1=y)
        nc.vector.tensor_mul(out=tn, in0=tn, in1=gv)
        nc.vector.tensor_scalar(out=tn, in0=tn, scalar1=-0.5,
                                scalar2=1.5, op0=OP.mult, op1=OP.add)
        nc.vector.tensor_mul(out=Gt[:, 1:8:2], in0=y, in1=tn)
        psB = ps_sm.tile([128, 8], F32, tag="psO")
        nc.tensor.matmul(out=psB, lhsT=indt_t, rhs=Gt, start=True, stop=True)
        keep_warm(n=2)  # in-order blip: paces right after psB executes
        # per-channel A = rstd*gn_scale ; B = gn_bias - mean*A
        Av = work.tile([128, 4], F32, tag="Av")
        nc.vector.tensor_mul(out=Av, in0=psB[:, 1:8:2], in1=gns_t)
        Aqk = work.tile([128, 4], F32, tag="Aqk")
        nc.vector.tensor_scalar_mul(out=Aqk, in0=Av, scalar1=QKS)
        # scaled weights (qk first: they gate the first projections)
        # split DVE/ACT so the serial chain before the first projection is
        # half as deep (ACT activation scale= is a per-partition vector)
        wqk_s, wv_s = [], []
        for k in range(4):
            a = const.tile([128, 128], BF16, tag=f"wqks{k}")
            if k < 2:
                nc.vector.tensor_scalar_mul(out=a, in0=wqk_raw[k],
                                            scalar1=Aqk[:, k:k + 1])
            else:
                nc.scalar.activation(out=a, in_=wqk_raw[k], func=AF.Identity,
                                     scale=Aqk[:, k:k + 1])
            wqk_s.append(a)
        for k in range(4):
            v = const.tile([128, CH], BF16, tag=f"wvs{k}")
            nc.scalar.activation(out=v, in_=wv_raw[k], func=AF.Identity,
                                 scale=Av[:, k:k + 1])
            wv_s.append(v)
        # bias-fold rhs vectors (off the proj critical path)
        t128 = work.tile([128, 4], F32, tag="t128")
        nc.vector.tensor_mul(out=t128, in0=psB[:, 0:8:2], in1=Av)
        Bc = work.tile([128, 4], F32, tag="Bc")
        nc.vector.tensor_sub(out=Bc, in0=gnb_t, in1=t128)
        Br = work.tile([128, 4], BF16, tag="Br")
        nc.vector.tensor_copy(out=Br, in_=Bc)
        Bsr = work.tile([128, 4], BF16, tag="Bsr")
        nc.vector.tensor_scalar_mul(out=Bsr, in0=Bc, scalar1=QKS)

        # ---------------- phase B: projections ----------------
        qk1 = big.tile([128, T], BF16, tag="qk1")   # [Q; K]
        kq2 = big.tile([128, T], BF16, tag="kq2")   # [K; Q] (DMA row-swap)
        vts = big.tile([CH, T], BF16, tag="vts")    # V^T (channel-major)
        # V token-major + ones col: bf16 for DVE-exp'd blocks, fp8e4 paired
        # (DoubleRow) for ACT-exp'd blocks
        vt = big.tile([128, NSB, VTP], BF16, tag="vt")
        nc.vector.memset(vt[:, :, CH:CH + 1], 1.0)
        vt8 = big.tile([128, NSB // 2, 2, VTP], FP8, tag="vt8")
        nc.vector.memset(vt8[:, :, :, CH:CH + 1], 1.0)

        def proj_qk(cc):
            sl = bass.ts(cc, TC)
            ps1 = ps_pair.tile([128, 1024], F32, name=f"qk_{cc}", tag="pp")
            for k in range(4):
                nc.tensor.matmul(out=ps1[:, 0:512], lhsT=wqk_s[k],
                                 rhs=xtk[k][:, sl], start=(k == 0),
                                 stop=(k == 3))
            nc.scalar.activation(out=qk1[:, sl], in_=ps1[:, 0:512],
                                 func=AF.Identity, bias=bqk_dev, scale=1.0)
            # kq2 = qk1 with row halves swapped. SBUF->SBUF copy on the
            # GPSIMD software-DGE queue: separate from the main HW DMA
            # queue (no head-of-line blocking) and costs no PE/DVE/ACT.
            nc.gpsimd.dma_start(out=kq2[0:64, sl], in_=qk1[64:128, sl])
            nc.gpsimd.dma_start(out=kq2[64:128, sl], in_=qk1[0:64, sl])

        # 64x64 identity for the PE transposes of V^T
        ident64 = const.tile([CH, CH], BF16, tag="ident64")
        from concourse.masks import make_identity
        make_identity(nc, ident64)

        def v_chunk(vc):
            # V^T projection chunk + PE transposes into token-major vt
            # (DMA XBAR transposes measured ~17GB/s -- far too slow)
            sl = bass.ts(vc, TC)
            psV = ps_pair.tile([128, 1024], F32, name=f"v_{vc}", tag="pp")
            for k in range(4):
                nc.tensor.matmul(out=psV[0:CH, 0:512], lhsT=wv_s[k],
                                 rhs=xtk[k][:, sl], start=(k == 0),
                                 stop=(k == 3))
            nc.scalar.activation(out=vts[:, sl], in_=psV[0:CH, 0:512],
                                 func=AF.Identity, bias=bv_dev, scale=1.0)
            # 4 PE transposes land in ONE psum tile; DVE then evacuates a
            # tile-pair (2 blocks) per copy instead of one block per copy
            psTrQ = ps_sm.tile([128, 4, CH], BF16, tag="psO")
            for j, ti in enumerate(range(4 * vc, 4 * vc + 4)):
                nc.tensor.transpose(out=psTrQ[:, j, :],
                                    in_=vts[:, bass.ts(ti, 128)],
                                    identity=ident64)
            for t in (2 * vc, 2 * vc + 1):
                j0 = (t - 2 * vc) * 2
                if t in ACT_TILES:
                    nc.vector.tensor_copy(out=vt8[:, t, :, 0:CH],
                                          in_=psTrQ[:, j0:j0 + 2, :])
                else:
                    nc.vector.tensor_copy(out=vt[:, 2 * t:2 * t + 2, 0:CH],
                                          in_=psTrQ[:, j0:j0 + 2, :])

        # first two qk projections before the bias folds (weights are ready
        # earlier than the folded biases; epilogues wait on bqk_dev anyway)
        # folded biases: b'' = s*b_host + (s*B) @ W_raw  (qk) / bv + B @ Wv
        psQKb = ps_sm.tile([128, 1], F32, tag="psO")
        for k in range(4):
            nc.tensor.matmul(out=psQKb, lhsT=wqk_raw[k],
                             rhs=Bsr[:, k:k + 1],
                             start=(k == 0), stop=(k == 3))
        bqk_dev = const.tile([128, 1], F32, tag="bqk_dev")
        nc.vector.tensor_add(out=bqk_dev, in0=psQKb[:, 0:1], in1=bqk_t)
        psBvT = ps_sm.tile([CH, 1], F32, tag="psO")
        for k in range(4):
            nc.tensor.matmul(out=psBvT, lhsT=wv_raw[k],
                             rhs=Br[:, k:k + 1],
                             start=(k == 0), stop=(k == 3))
        bv_dev = const.tile([CH, 1], F32, tag="bv_dev")
        nc.vector.tensor_add(out=bv_dev, in0=psBvT[:, 0:1], in1=bvT_t)

        # interleaved qk1/V projections: qk1 first (it gates attention),
        # V chunks fill the stream so vt transposes land before mm2 needs
        # them. All on a warm PE right after the fold.
        proj_qk(0)
        proj_qk(1)
        for c in range(NTC - 2):
            proj_qk(c + 2)
            v_chunk(c)
        v_chunk(NTC - 2)
        v_chunk(NTC - 1)

        # ---------------- phase C: attention ----------------
        outsb = big.tile([CH, T], F32, tag="outsb")
        NT_C = 16  # tiles of 2 s-blocks; tile == mm1 row-pair

        rb_dram = nc.dram_tensor("rb_dram", [NTC, TC], F32)

        def epi_start(state):
            cc0, psO0 = state
            den = work.tile([1, TC], F32, name=f"den_{cc0}", tag="den")
            nc.vector.tensor_copy(out=den, in_=psO0[CH:CH + 1, :])
            if cc0 == NTC - 1:
                # LAST chunk only: its epilogue chain is fully serial (no
                # next chunk hides it), so use the free-size reciprocal on
                # the [1,512] row directly -- skips the [128,4] reshape
                # bounce (one DMA hop + recip shorter). Mid-stream chunks
                # keep the reshape path: the custom DVE op between exp
                # batches measurably slowed the 12.0us chunk cadence (v11).
                rq1 = work.tile([1, TC], F32, name=f"rq1_{cc0}", tag="rq1")
                nc.vector.reciprocal_approx_fast(out=rq1, in_=den)
                nc.sync.dma_start(out=rb_dram[cc0:cc0 + 1, :], in_=rq1)
                return None
            rsm = work.tile([128, 4], F32, name=f"rsm_{cc0}", tag="rsm")
            nc.sync.dma_start(out=rsm, in_=den)
            return rsm

        def epi_finish(state, rsm):
            cc0, psO0 = state
            sl0 = bass.ts(cc0, TC)
            if rsm is not None:
                rq = work.tile([128, 4], F32, name=f"rq_{cc0}", tag="rq")
                nc.vector.reciprocal(out=rq, in_=rsm)
                nc.sync.dma_start(out=rb_dram[cc0:cc0 + 1, :], in_=rq)
            row = rb_dram[cc0]
            bc = bass.AP(tensor=row.tensor, offset=row.offset,
                         ap=[[0, CH]] + [list(a) for a in row.ap])
            rb = work.tile([CH, TC], F32, name=f"rb_{cc0}", tag="rb")
            nc.sync.dma_start(out=rb, in_=bc)
            nc.vector.tensor_mul(out=outsb[:, sl0], in0=psO0[0:CH, :], in1=rb)
            nc.sync.dma_start(out=outT[:, 512 * cc0:512 * (cc0 + 1)],
                              in_=outsb[:, sl0])

        def mm1_pair(cc, pr, pp):
            sl = bass.ts(cc, TC)
            for half, blk in ((0, 2 * pr), (1, 2 * pr + 1)):
                dst = pp[:, 512 * half:512 * (half + 1)]
                if half == 0:
                    nc.tensor.matmul(out=dst,
                                     lhsT=kq2[0:64, bass.ts(blk, 128)],
                                     rhs=qk1[0:64, sl], start=True, stop=True,
                                     tile_position=(0, 0))
                else:
                    nc.tensor.matmul(out=dst,
                                     lhsT=qk1[64:128, bass.ts(blk, 128)],
                                     rhs=kq2[64:128, sl], start=True,
                                     stop=True, tile_position=(64, 0))

        def exp_tile(pp, ti):
            if ti in ACT_TILES:
                # exp(x)/16 straight to fp8e4, plane-paired for DoubleRow
                P = ppool.tile([128, 2, 512], FP8, tag="P8")
                nc.scalar.activation(out=P, in_=pp, func=AF.Exp,
                                     bias=ln16b, scale=1.0)
            else:
                P = ppool.tile([128, 1024], BF16, tag="P")
                # per-512 slices: a DVE PSUM read must stay within one bank.
                # bank 1 first so bank 0 frees LAST (see v2 notes).
                # NOTE: splitting one tile's banks across ACT+DVE for
                # balance measured SLOWER (ACT's extra half lands late in
                # the chunk and delays PSUM slot recycling) -- keep 9/7.
                for c in (1, 0):
                    cs = slice(512 * c, 512 * (c + 1))
                    nc.vector.tensor_scalar(out=P[:, cs].bitcast(I16),
                                            in0=pp[:, cs], scalar1=SCH_A,
                                            scalar2=SCH_B, op0=OP.mult,
                                            op1=OP.add)
            return P

        def mm2_tile(psO, ti, P, first, last):
            if ti in ACT_TILES:
                # one DoubleRow matmul covers both s-blocks (K=256 eff.)
                # NOTE: M=128 zero-padded DR weights measured SLOWER (the
                # 256-col LDWEIGHTS doesn't hide) -- keep M=65.
                nc.tensor.matmul(out=psO[0:CH + 1, :],
                                 lhsT=vt8[:, ti, :, 0:CH + 1],
                                 rhs=P, perf_mode=DR,
                                 start=first, stop=last)
                return
            for c in range(2):
                blk = 2 * ti + c
                nc.tensor.matmul(out=psO[0:CH + 1, :],
                                 lhsT=vt[:, blk, 0:CH + 1],
                                 rhs=P[:, 512 * c:512 * (c + 1)],
                                 start=(first and c == 0),
                                 stop=(last and c == 1))

        pending = None
        pending_rsm = None
        for cc in range(NTC):
            psO = ps_o.tile([128, TC], F32, tag="psO")
            # batched streams: 3 mm1 pairs back-to-back, their exps, then
            # the PREVIOUS batch's mm2s. During chunk 0 the V projection
            # chunks are interleaved into the pair stream (one per 2 pairs).
            batch = []        # (ti, pp) awaiting exp
            prev_p = []       # (ti, P) awaiting mm2
            for pr in range(NT_C):
                pp = ps_pair.tile([128, 1024], F32, name=f"pp_{cc}_{pr}",
                                  tag="pp")
                mm1_pair(cc, pr, pp)
                batch.append((pr, pp))
                if pending is not None and pr == 0:
                    pending_rsm = epi_start(pending)
                if len(batch) == 3 or pr == NT_C - 1:
                    newp = [(ti, exp_tile(tp, ti)) for ti, tp in batch]
                    batch = []
                    for ti, P in prev_p:
                        mm2_tile(psO, ti, P, first=(ti == 0), last=False)
                    prev_p = newp
                if pending is not None and pr == 2:
                    epi_finish(pending, pending_rsm)
                    pending = None
                    pending_rsm = None
            for j, (ti, P) in enumerate(prev_p):
                mm2_tile(psO, ti, P, first=False, last=(j == len(prev_p) - 1))
            pending = (cc, psO)
        rsm = epi_start(pending)
        epi_finish(pending, rsm)


_NC_CACHE = None
TRACE = False          # set True (e.g. from test.py) to capture an NTFF profile
TMPDIR = None          # set to a path to keep NEFF/NTFF artifacts
LAST_RESULT = None     # BassKernelResults of the most recent kernel() call


def _get_nc():
    global _NC_CACHE
    if _NC_CACHE is None:
        _NC_CACHE = build_nc()
    return _NC_CACHE


def _to_bf16(a):
    import ml_dtypes
    return np.ascontiguousarray(a).astype(ml_dtypes.bfloat16)


def kernel(x, gn_scale, gn_bias, w_qkv, b_qkv):
    x = np.asarray(x, dtype=np.float32)
    gn_scale = np.asarray(gn_scale, dtype=np.float32)
    gn_bias = np.asarray(gn_bias, dtype=np.float32)
    w_qkv = np.asarray(w_qkv, dtype=np.float32)
    b_qkv = np.asarray(b_qkv, dtype=np.float32)

    spatial = x.shape[:-1]
    xt = _to_bf16(x.reshape(T, C).T)                         # (C, T) bf16
    w3 = w_qkv.reshape(C, C, 3)
    wq, wk, wv = w3[..., 0], w3[..., 1], w3[..., 2]          # (C, C) each
    b3 = b_qkv.reshape(C, 3)
    bq, bk, bvv = b3[:, 0], b3[:, 1], b3[:, 2]

    # indicator matrices: channel partition p -> group j = p // 16
    p_idx = np.arange(128)
    ind = (p_idx[:, None] // 16 == np.arange(8)[None, :]).astype(np.float32)
    indt = np.ascontiguousarray(ind.T)
    gns_dev = np.ascontiguousarray(gn_scale.reshape(4, 128).T)
    gnb_dev = np.ascontiguousarray(gn_bias.reshape(4, 128).T)

    in_maps = []
    for h in range(H):
        hsl = slice(CH * h, CH * (h + 1))
        wq_h, wk_h, wv_h = wq[:, hsl], wk[:, hsl], wv[:, hsl]
        in_maps.append({
            "xt": xt,
            "wqk": _to_bf16(np.concatenate([wq_h, wk_h], axis=1)),
            "wv": _to_bf16(wv_h),
            "bqk": (QKS * np.concatenate([bq[hsl], bk[hsl]])).reshape(128, 1),
            "bvT": bvv[hsl].reshape(CH, 1).copy(),
            "gns": gns_dev,
            "gnb": gnb_dev,
            "ind": ind,
            "indt": indt,
        })

    nc = _get_nc()
    res = run_bass_kernel_spmd(nc, in_maps, list(range(NCORES)), trace=TRACE,
                               tmpdir=TMPDIR)
    global LAST_RESULT
    LAST_RESULT = res
    out = np.empty((T, C), dtype=np.float32)
    for h in range(H):
        out[:, CH * h:CH * (h + 1)] = res.results[h]["outT"].T
    return out.reshape(spatial + (C,))
